# revision 1
# baseline (speedup 1.0000x reference)
"""Dynamic (MoE-routed) 3x3 conv kernel for Trainium2, 8 NeuronCores.

Problem: nn_DynamicConv_670014898566
  x         [32, 64, 128, 128] f32
  w_route   [4, 64] f32
  b_route   [4] f32
  w_experts [4, 64, 64, 3, 3] f32
  y = per-sample conv2d(x, sigmoid(mean(x,HW) @ w_route.T + b_route) @ w_experts, SAME)

Sharding: data-parallel over batch, 4 samples per core (2 pairs of 2).

Per-core device program (Tile framework):
  - x loaded as bf16 via SWDGE cast-DMA eighths on the gpsimd ring (the only
    load path that sustains ~330 GB/s; HWDGE rings stall ~2us between DMAs)
  - routing channel-sums per eighth as each lands: DVE reduce / ACT
    activation-accum for pair 0, gpsimd reduces for pair 1 (the gpsimd queue
    is idle during conv, so pair-1 prep can never back-pressure the conv)
  - expert kernels PE-transposed to lhsT layout ONCE at startup while loads
    stream; per-pair kernel mix is 4 chained ops in bf16 split column-wise
    across DVE and gpsimd
  - routing: masked pooled columns -> one f32 matmul for both samples'
    logits -> bias add -> sigmoid -> broadcast matmul over partitions
  - conv: per (sample h, chunk-parity q) stream, 9 shifted bf16 matmuls
    accumulate into one PSUM region; 4-way PE tile parallelism (64h, 64q)
  - pair-1 chain ops that need ACT/PE/DVE are spliced between conv pair-0
    t-groups late enough that their inputs have always landed
  - y stored as bf16 (halves write traffic) in 512KB DMAs split across the
    two free HWDGE rings; host upcasts to f32
"""

import sys

sys.path.insert(0, "/opt/trn_rl_repo")

import numpy as np

B, C, H, W = 32, 64, 128, 128
E = 4
HW = H * W
N_CORES = 8
NS = B // N_CORES          # samples per core = 4
NPAIR = NS // 2            # pairs per core = 2
NE = 8                     # load eighths per pair
ECOLS = HW // NE           # 2048 cols per eighth (pair layout: 128 part x 16384)
NT = 16                    # chunk-pairs (t) per pair
NSG = 2                    # store super-groups per pair (4 g of 2 t each)
MIXSPLIT = 288             # mix column split: DVE [0:288), gpsimd [288:576)
# full-coverage tap first (owns start=True so PSUM has_written covers the bank)
TAPS = [(1, 1), (0, 0), (0, 1), (0, 2), (1, 0), (1, 2), (2, 0), (2, 1), (2, 2)]

_CACHE = {}


def _build_nc():
    import concourse.bacc as bacc
    import concourse.mybir as mybir
    import concourse.tile as tile

    dt = mybir.dt
    f32 = dt.float32
    bf16 = dt.bfloat16

    nc = bacc.Bacc("TRN2", target_bir_lowering=False, debug=False, num_devices=N_CORES)

    x_d = nc.dram_tensor("x", [NS, C, H, W], f32, kind="ExternalInput")
    wr_d = nc.dram_tensor("w_route", [E, C], f32, kind="ExternalInput")
    br_d = nc.dram_tensor("b_route", [E], f32, kind="ExternalInput")
    we_d = nc.dram_tensor("w_experts", [E, C, C, 3, 3], f32, kind="ExternalInput")
    y_d = nc.dram_tensor("y", [NS, C, H, W], bf16, kind="ExternalOutput")

    # x viewed as [(b c), (h w)]: pair p = rows 128p..128p+128
    x_flat = x_d.ap().rearrange("b c h w -> (b c) (h w)")
    # y viewed as [b, c, SG, g4, t2, parity, 4*W] for batched stores
    y_g = y_d.ap().rearrange(
        "b c (G g4 t2 hf r) w -> b c G g4 t2 hf (r w)", G=NSG, g4=4, t2=2, hf=2, r=4
    )
    # w_experts as [o, (e, c*9)] - all experts on one partition block
    we_all = we_d.ap().rearrange("e o c kh kw -> o e (c kh kw)")

    with tile.TileContext(nc) as tc:
        with (
            tc.tile_pool(name="const", bufs=1) as cpool,
            tc.tile_pool(name="xp", bufs=2) as xpool,
            tc.tile_pool(name="mix", bufs=2) as mpool,
            tc.tile_pool(name="small", bufs=2) as spool,
            tc.tile_pool(name="stage", bufs=4) as stpool,
            tc.tile_pool(name="cps", bufs=6, space="PSUM") as convps,
            tc.tile_pool(name="trps", bufs=1, space="PSUM") as trps,
            tc.tile_pool(name="rps", bufs=1, space="PSUM") as rps,
        ):
            xb_t = [
                xpool.tile([128, HW], bf16, tag="xt", name=f"xb_p{p}")
                for p in range(NPAIR)
            ]
            pooled_t = [
                spool.tile([128, NE + 1], f32, tag="pooled", name=f"pooled_{p}")
                for p in range(NPAIR)
            ]

            # ---------------- gpsimd: ident, then pair-0 cast loads ----------
            ident = cpool.tile([C, C], f32)
            nc.gpsimd.memset(ident[:], 1.0)
            nc.gpsimd.affine_select(
                out=ident[:], in_=ident[:],
                compare_op=mybir.AluOpType.is_equal, fill=0.0,
                base=0, pattern=[[-1, C]], channel_multiplier=1,
            )

            def emit_load(p, qu):
                nc.gpsimd.dma_start(
                    xb_t[p][:, qu * 2 * ECOLS : (qu + 1) * 2 * ECOLS],
                    x_flat[
                        128 * p : 128 * p + 128, qu * 2 * ECOLS : (qu + 1) * 2 * ECOLS
                    ],
                )

            for qu in range(4):
                emit_load(0, qu)

            # masks between the two pairs' trigger batches
            # mask2[s, p] = 1 iff p//64 == s (for routing broadcast over parts)
            mask2 = cpool.tile([2, 128], f32)
            nc.gpsimd.memset(mask2[:], 1.0)
            nc.gpsimd.affine_select(
                out=mask2[:], in_=mask2[:],
                compare_op=mybir.AluOpType.is_ge, fill=0.0,
                base=0, pattern=[[1, 128]], channel_multiplier=-64,
            )
            nc.gpsimd.affine_select(
                out=mask2[:], in_=mask2[:],
                compare_op=mybir.AluOpType.is_ge, fill=0.0,
                base=63, pattern=[[-1, 128]], channel_multiplier=64,
            )
            # mask01[p, s] = 1 iff p//64 == s (to split pooled col by sample)
            mask01 = cpool.tile([128, 2], f32)
            nc.gpsimd.memset(mask01[:], 0.0)
            nc.gpsimd.memset(mask01[0:64, 0:1], 1.0)
            nc.gpsimd.memset(mask01[64:128, 1:2], 1.0)

            for qu in range(4):
                emit_load(1, qu)

            # ---------------- small constant loads: scalar (ACT) ring ---------
            we_sb = cpool.tile([C, E * C * 9], f32)
            nc.scalar.dma_start(we_sb[:], we_all)
            wr_raw = cpool.tile([E, C], f32)
            nc.scalar.dma_start(wr_raw[:], wr_d.ap())
            bias_rep = cpool.tile([2, E], f32)
            br_row = br_d.ap().rearrange("(one e) -> one e", one=1)
            nc.scalar.dma_start(bias_rep[0:1, :], br_row)
            nc.scalar.dma_start(bias_rep[1:2, :], br_row)

            # ---------------- startup PE work: expert weights -> lhsT ---------
            # weT[c(+64h), e*576 + tap*64 + o], bf16, both halves identical.
            # Transposes go first in the PE FIFO (inputs ready early); the
            # PSUM->SBUF copies are interleaved with pair 0's reduces below.
            weT = cpool.tile([128, E * C * 9], bf16)
            we_r = we_sb.rearrange("p (e c t) -> p e c t", e=E, t=9)
            tr_tiles = []
            for ei in range(E):
                for r0, r1 in ((0, 5), (5, 9)):
                    tr = trps.tile(
                        [C, (r1 - r0) * C], f32, tag="tr", name=f"tr_{ei}_{r0}"
                    )
                    for tap in range(r0, r1):
                        nc.tensor.transpose(
                            tr[:, (tap - r0) * C : (tap - r0 + 1) * C],
                            we_r[:, ei, :, tap],
                            ident[:],
                        )
                    tr_tiles.append(
                        (tr, weT[0:C, ei * 576 + r0 * C : ei * 576 + r1 * C])
                    )
            # routing matrix wrT_rep[c(+64h), e] = w_route.T / HW, replicated
            wrT_rep = cpool.tile([128, E], f32)
            wr_ps = rps.tile([C, E], f32, tag="rps", name="wr_ps")
            nc.tensor.transpose(wr_ps[:], wr_raw[:], ident[0:E, 0:E])

            # ---------------- per-pair prep helpers ----------------
            act_scratch = cpool.tile([128, ECOLS], bf16)

            def emit_red_dve(p, e):
                nc.vector.reduce_sum(
                    pooled_t[p][:, e : e + 1],
                    xb_t[p][:, e * ECOLS : (e + 1) * ECOLS],
                    axis=mybir.AxisListType.X,
                )

            def emit_red_act(p, e):
                nc.scalar.activation(
                    act_scratch[:], xb_t[p][:, e * ECOLS : (e + 1) * ECOLS],
                    mybir.ActivationFunctionType.Copy,
                    accum_out=pooled_t[p][:, e : e + 1],
                )

            def emit_tail_masked(p, eng):
                pooled = pooled_t[p]
                eng.reduce_sum(
                    pooled[:, NE : NE + 1], pooled[:, 0:NE], axis=mybir.AxisListType.X
                )
                masked = spool.tile([128, 2], f32, tag="masked", name=f"masked_{p}")
                eng.tensor_scalar_mul(masked[:], mask01[:], pooled[:, NE : NE + 1])
                return masked

            def emit_logits(p, masked):
                logits_ps = rps.tile([2, E], f32, tag="rps", name=f"lg_{p}")
                nc.tensor.matmul(logits_ps[:], masked[:], wrT_rep[:])
                return logits_ps

            def emit_bias(p, logits_ps):
                logits_sb = spool.tile([2, E], f32, tag="lsb", name=f"lsb_{p}")
                nc.vector.tensor_tensor(
                    logits_sb[:], logits_ps[:], bias_rep[:], mybir.AluOpType.add
                )
                return logits_sb

            def emit_sig(p, logits_sb):
                rT = spool.tile([2, E], f32, tag="rT", name=f"rT_{p}")
                nc.scalar.activation(
                    rT[:], logits_sb[:], mybir.ActivationFunctionType.Sigmoid
                )
                return rT

            def emit_bcast(p, rT):
                rbc_ps = rps.tile([128, E], f32, tag="rps", name=f"rb_{p}")
                nc.tensor.matmul(rbc_ps[:], mask2[:], rT[:])
                return rbc_ps

            def emit_mix(p, rbc_ps):
                # wmixT[c(+64h), tap*64+o] = sum_e r[h, e] * weT[., e, .] (bf16)
                mixa = mpool.tile([128, C * 9], bf16, tag="mixa", name=f"mixa_{p}")
                mixb = mpool.tile([128, C * 9], bf16, tag="mixb", name=f"mixb_{p}")
                nc.vector.tensor_scalar_mul(mixa[:], weT[:, 0:576], rbc_ps[:, 0:1])
                nc.vector.scalar_tensor_tensor(
                    mixb[:], weT[:, 576:1152], rbc_ps[:, 1:2], mixa[:],
                    op0=mybir.AluOpType.mult, op1=mybir.AluOpType.add,
                )
                nc.vector.scalar_tensor_tensor(
                    mixa[:], weT[:, 1152:1728], rbc_ps[:, 2:3], mixb[:],
                    op0=mybir.AluOpType.mult, op1=mybir.AluOpType.add,
                )
                nc.vector.scalar_tensor_tensor(
                    mixb[:], weT[:, 1728:2304], rbc_ps[:, 3:4], mixa[:],
                    op0=mybir.AluOpType.mult, op1=mybir.AluOpType.add,
                )
                return mixb

            # ---------------- pair 0 prep, interleaved with weT copies --------
            # DVE takes even eighths, ACT odd (the final eighth lands on ACT);
            # weT copies fill the gaps (only needed by the mix at chain end).
            # weT copies + replicates go FIRST on their FIFOs: the pair-0
            # reduces are load-gated (only q3's reduce is on the critical
            # path, at ~land+2.3us), while the mix needs weT complete by then.
            # Replicates ride the sync HWDGE ring (empty until conv stores),
            # per expert, so each fires the moment its copies finish.
            sig_warm = cpool.tile([1, 1], f32)
            nc.scalar.activation(
                sig_warm[:], bias_rep[0:1, 0:1],
                mybir.ActivationFunctionType.Sigmoid,
            )
            nc.scalar.mul(wrT_rep[0:C, :], wr_ps[:], 1.0 / HW)
            nc.sync.dma_start(wrT_rep[C : 2 * C, :], wrT_rep[0:C, :])
            for ei in range(E):
                nc.vector.tensor_copy(tr_tiles[2 * ei][1], tr_tiles[2 * ei][0][:])
                nc.scalar.copy(tr_tiles[2 * ei + 1][1], tr_tiles[2 * ei + 1][0][:])
                nc.sync.dma_start(
                    weT[C : 2 * C, ei * 576 : (ei + 1) * 576],
                    weT[0:C, ei * 576 : (ei + 1) * 576],
                )
            for qu in range(4):
                emit_red_dve(0, 2 * qu)
                emit_red_act(0, 2 * qu + 1)

            masked0 = emit_tail_masked(0, nc.vector)
            logits0 = emit_logits(0, masked0)
            lsb0 = emit_bias(0, logits0)
            rT0 = emit_sig(0, lsb0)
            rbc0 = emit_bcast(0, rT0)
            wmixT_t = [emit_mix(0, rbc0), None]

            # pair-1 prep is spliced between conv pair-0 t-groups at positions
            # late enough that each op's inputs have landed before its engine
            # FIFO reaches it (a stalled splice op would block the conv copies
            # queued behind it). Evens on DVE, odds on ACT.
            p1 = {}
            P1_RED = {0: 0, 2: 1, 6: 2, 11: 3}  # conv-p0 t -> pair-1 quarter

            def splice_p1(t):
                qu = P1_RED.get(t)
                if qu is not None:
                    emit_red_dve(1, 2 * qu)
                    emit_red_act(1, 2 * qu + 1)
                if t == 11:
                    p1["masked"] = emit_tail_masked(1, nc.vector)
                elif t == 12:
                    p1["lsb"] = emit_bias(1, emit_logits(1, p1["masked"]))
                    p1["rT"] = emit_sig(1, p1["lsb"])
                elif t == 13:
                    wmixT_t[1] = emit_mix(1, emit_bcast(1, p1["rT"]))

            # ---------------- conv ----------------
            for p in range(NPAIR):
                conv_scope = nc.named_scope(f"conv_p{p}"); conv_scope.__enter__()
                xb = xb_t[p]
                xb3 = xb.rearrange("p_ (r c) -> p_ r c", c=W)
                for sg in range(NSG):
                    stA = stpool.tile(
                        [128, 4, 2, 512], bf16, tag="stage", name=f"stA_{p}_{sg}"
                    )
                    stB = stpool.tile(
                        [128, 4, 2, 512], bf16, tag="stage", name=f"stB_{p}_{sg}"
                    )
                    for g4 in range(4):
                        for tg in range(2):
                            t = 8 * sg + 2 * g4 + tg
                            wmixT = wmixT_t[p]
                            psA = convps.tile(
                                [128, 512], f32, tag="cps", name=f"psA_{p}_{t}"
                            )
                            psB = convps.tile(
                                [128, 512], f32, tag="cps", name=f"psB_{p}_{t}"
                            )
                            psA3 = psA.rearrange("p_ (r c) -> p_ r c", c=W)
                            psB3 = psB.rearrange("p_ (r c) -> p_ r c", c=W)
                            # stream (h, q) -> psum region: (0,0)->psA[0:64],
                            # (1,1)->psA[64:], (1,0)->psB[0:64], (0,1)->psB[64:]
                            for tap_idx, (kh, kw) in enumerate(TAPS):
                                cstart = max(0, 1 - kw)
                                cend = min(W, W + 1 - kw)
                                ncols = cend - cstart
                                ic0 = cstart + kw - 1
                                for h in range(2):
                                    for q in range(2):
                                        ps3 = psA3 if h == q else psB3
                                        j = 2 * t + q
                                        rstart = max(4 * j, 1 - kh)
                                        rend = min(4 * j + 4, H + 1 - kh)
                                        nrows = rend - rstart
                                        ir0 = rstart + kh - 1
                                        nc.tensor.matmul(
                                            ps3[
                                                64 * q : 64 * q + 64,
                                                rstart - 4 * j : rstart - 4 * j + nrows,
                                                cstart:cend,
                                            ],
                                            wmixT[
                                                64 * h : 64 * h + 64,
                                                (3 * kh + kw) * 64
                                                : (3 * kh + kw) * 64 + 64,
                                            ],
                                            xb3[
                                                64 * h : 64 * h + 64,
                                                ir0 : ir0 + nrows,
                                                ic0 : ic0 + ncols,
                                            ],
                                            start=(tap_idx == 0),
                                            stop=(tap_idx == len(TAPS) - 1),
                                        )
                            # stA on ACT, stB on DVE (split so both keep up)
                            nc.scalar.copy(stA[:, g4, tg, :], psA[:])
                            nc.vector.tensor_copy(stB[:, g4, tg, :], psB[:])
                            if p == 0:
                                splice_p1(t)
                            if p == NPAIR - 1 and sg == NSG - 1 and g4 == 3:
                                # very last chunks: store per tg so the final
                                # DMA is small and the kernel tail shrinks
                                bA, bB = 2 * p, 2 * p + 1
                                nc.sync.dma_start(
                                    y_g[bA, :, sg, g4, tg, 0, :],
                                    stA[0:64, g4, tg, :],
                                )
                                nc.sync.dma_start(
                                    y_g[bA, :, sg, g4, tg, 1, :],
                                    stB[64:128, g4, tg, :],
                                )
                                nc.scalar.dma_start(
                                    y_g[bB, :, sg, g4, tg, 0, :],
                                    stB[0:64, g4, tg, :],
                                )
                                nc.scalar.dma_start(
                                    y_g[bB, :, sg, g4, tg, 1, :],
                                    stA[64:128, g4, tg, :],
                                )
                        if p == NPAIR - 1 and sg == NSG - 1 and g4 == 3:
                            continue
                        # stores flow per g4 (128KB DMAs) right behind the
                        # copies: A-sample on sync ring, B-sample on scalar.
                        # stage: stA = [A even chunks; B odd], stB = [B even; A odd]
                        bA, bB = 2 * p, 2 * p + 1
                        nc.sync.dma_start(
                            y_g[bA, :, sg, g4, :, 0, :], stA[0:64, g4, :, :]
                        )
                        nc.sync.dma_start(
                            y_g[bA, :, sg, g4, :, 1, :], stB[64:128, g4, :, :]
                        )
                        nc.scalar.dma_start(
                            y_g[bB, :, sg, g4, :, 0, :], stB[0:64, g4, :, :]
                        )
                        nc.scalar.dma_start(
                            y_g[bB, :, sg, g4, :, 1, :], stA[64:128, g4, :, :]
                        )
                conv_scope.__exit__(None, None, None)

    nc.compile()
    return nc


def _get_nc():
    if "nc" not in _CACHE:
        _CACHE["nc"] = _build_nc()
    return _CACHE["nc"]


def _run(inputs, trace=False, **kw):
    from concourse import bass_utils

    nc = _get_nc()
    x = np.ascontiguousarray(inputs["x"], dtype=np.float32)
    in_maps = [
        {
            "x": x[i * NS : (i + 1) * NS],
            "w_route": np.ascontiguousarray(inputs["w_route"], dtype=np.float32),
            "b_route": np.ascontiguousarray(inputs["b_route"], dtype=np.float32),
            "w_experts": np.ascontiguousarray(inputs["w_experts"], dtype=np.float32),
        }
        for i in range(N_CORES)
    ]
    res = bass_utils.run_bass_kernel_spmd(
        nc, in_maps, core_ids=list(range(N_CORES)), trace=trace, **kw
    )
    y = np.concatenate(
        [np.asarray(res.results[i]["y"]).astype(np.float32) for i in range(N_CORES)],
        axis=0,
    )
    return y, res


def kernel(**inputs):
    y, _ = _run(inputs)
    return y

